# revision 64
# baseline (speedup 1.0000x reference)
"""AttentionBlock (GroupNorm + 1x1-conv qkv + MHA + proj + residual) on 8 trn2 cores.

Sharding: data-parallel over batch (B=8 -> 1 batch item per core); weights
replicated. Each core computes its full attention block on x[b] in [C, L]
layout (C=512 channels, L=1024 positions).

v2 design (vs the bf16 baseline):
  - qkv and AV matmuls run in fp8(e4m3) DoubleRow mode (2 fp8 weights/cell,
    contraction 256 per matmul) -> ~2x PE throughput on those stages. QK and
    proj stay bf16 (QK pairs already run 2x via row-tiling; proj keeps the
    j-split tail cheap and accurate).
  - softmax exp emits fp8 P directly: exp(s - 3) (offset keeps max ~30 << 240
    e4m3 limit; offset cancels between numerator and ones-column denominator).
  - groupnorm stats all on DVE (bn_stats), so ACT only ever loads one table
    set and the exp stream can start ~10us in (vs ~29us).
  - proj t-half epilogues batched into single 1MB output DMAs (one per half)
    on otherwise-idle engine rings (gpsimd / scalar) to kill the DMA drain
    tail.
  - last unit's softmax-denominator reciprocal runs on ACT (ln/exp) + a K=1
    ones-matmul partition-broadcast, skipping two DRAM round trips on the
    critical tail; earlier units keep the packed-DVE-reciprocal DRAM bounce
    (fully hidden mid-stream).

Host-side preprocessing (exact math, no approximation beyond dtype casts):
  - GroupNorm affine (gn_w, gn_b) folded into qkv_w/qkv_b.
  - Attention scale (1/sqrt(sqrt(ch)) on both q and k) folded into qkv rows.
  - qkv rows permuted to head-major [q_heads | k_heads | v_heads].
  - proj bias folded into the f32 residual copy of x (out = (x+proj_b) + pw@a).
  - Weights pre-transposed into the PE's stationary (lhsT = [K, M]) layout;
    fp8 weights pre-paired for DoubleRow ([K, jp, ko, M] with contraction
    channel (2*jp+ko)*128 + K).
"""

import math
import os
from contextlib import ExitStack

import ml_dtypes
import numpy as np

B, C, HH, WW = 8, 512, 32, 32
L = HH * WW          # 1024
NH = 8               # heads
NG = 32              # groupnorm groups
CH = C // NH         # 64 head dim
EPS = 1e-5
NCORES = 8
OFF = 3.0            # exp offset: p = exp(s - OFF), max ~e^3.4 ~ 30 << 240

_cache = {}


def _build_program():
    import concourse.bass as bass
    import concourse.mybir as mybir
    import concourse.tile as tile

    F32 = mybir.dt.float32
    BF16 = mybir.dt.bfloat16
    F8 = mybir.dt.float8e4
    Alu = mybir.AluOpType
    Act = mybir.ActivationFunctionType
    DR = mybir.MatmulPerfMode.DoubleRow

    nc = bass.Bass()

    xb_d = nc.dram_tensor("xb", [C, L], BF16, kind="ExternalInput")
    x_d = nc.dram_tensor("x", [C, L], F32, kind="ExternalInput")
    # fp8 DoubleRow lhsT for q,k,v: [p, which(3), jp(2), ko(2), out(512)]
    w8_d = nc.dram_tensor("w8", [128, 3, 2, 2, C], F8, kind="ExternalInput")
    pw_d = nc.dram_tensor("pw", [128, 4, C], BF16, kind="ExternalInput")
    bv_d = nc.dram_tensor("bv", [1, C], BF16, kind="ExternalInput")
    # packed small constants: cols 0:8 g/16, 8:16 [bq|bk], rows 0:8 of
    # cols 16:144 hold gt, cols 144:152 g/(16*L) (for the ACT sum-stats path)
    gg_d = nc.dram_tensor("gg", [128, 160], F32, kind="ExternalInput")
    out_d = nc.dram_tensor("out", [C, L], F32, kind="ExternalOutput")
    # DRAM bounce buffers for the softmax denominators (units 0..6): rdram
    # collects the raw per-(head, t) sums, rdram2 the bf16 reciprocals packed
    # for the partition-broadcast reload.
    rdram = nc.dram_tensor("rdram", [NH, L], BF16)
    rdram2 = nc.dram_tensor("rdram2", [NH, L], BF16)

    xb_p = xb_d.rearrange("(n p) l -> p n l", p=128)
    x_p = x_d.rearrange("(n p) l -> p n l", p=128)
    out_p = out_d.rearrange("(n p) l -> p n l", p=128)

    with tile.TileContext(nc) as tc, ExitStack() as stack:
        const = stack.enter_context(tc.tile_pool(name="const", bufs=1))
        big = stack.enter_context(tc.tile_pool(name="big", bufs=1))
        small = stack.enter_context(tc.tile_pool(name="small", bufs=1))
        ptp = stack.enter_context(tc.tile_pool(name="ptp", bufs=3))
        aup = stack.enter_context(tc.tile_pool(name="aup", bufs=4))
        outp = stack.enter_context(tc.tile_pool(name="outp", bufs=3))
        rbcp = stack.enter_context(tc.tile_pool(name="rbcp", bufs=4))
        pkp = stack.enter_context(tc.tile_pool(name="pkp", bufs=4))
        tailp = stack.enter_context(tc.tile_pool(name="tailp", bufs=2))
        gn_stack = ExitStack()
        gn_ps = gn_stack.enter_context(tc.tile_pool(name="gn_ps", bufs=1, space="PSUM"))
        warm_stack = ExitStack()
        warm_ps = warm_stack.enter_context(
            tc.tile_pool(name="warm_ps", bufs=1, space="PSUM")
        )

        # ---- loads. sync ring carries only the startup-critical tensors in
        # need-order (xb -> stats; gg -> group reduce; w8 -> qkv matmuls);
        # everything else rides the gpsimd SWDGE ring so the sync triggers
        # (~0.7us each, serialized) don't push them out.
        xb_sb = []
        for jj in range(2):
            xbt = big.tile([128, 2, L], BF16, name=f"xb{jj}")
            nc.sync.dma_start(out=xbt, in_=xb_p[:, 2 * jj : 2 * jj + 2, :])
            xb_sb.append(xbt[:, 0, :])
            xb_sb.append(xbt[:, 1, :])
        gg_sb = const.tile([128, 160], F32, name="gg_sb")
        nc.sync.dma_start(out=gg_sb, in_=gg_d[:])
        w8_sb = const.tile([128, 3, 2, 2, C], F8, name="w8_sb")
        nc.sync.dma_start(out=w8_sb, in_=w8_d[:])
        bv_sb = const.tile([1, C], BF16, name="bv_sb")
        nc.gpsimd.dma_start(out=bv_sb, in_=bv_d[:])
        pw_sb = const.tile([128, 4, C], BF16, name="pw_sb")
        nc.gpsimd.dma_start(out=pw_sb, in_=pw_d[:])

        ones_col = const.tile([1, 128], BF16, name="ones_col")
        nc.vector.memset(ones_col, 1.0)
        ones64 = const.tile([1, CH], BF16, name="ones64")
        nc.vector.memset(ones64, 1.0)
        eps8 = const.tile([8, 1], F32, name="eps8")
        nc.vector.memset(eps8, EPS)
        zero8 = const.tile([8, 1], F32, name="zero8")
        nc.vector.memset(zero8, 0.0)
        zero1 = const.tile([1, 1], F32, name="zero1")
        nc.vector.memset(zero1, 0.0)
        noff = const.tile([128, 1], F32, name="noff")
        nc.vector.memset(noff, -OFF)
        zero128 = const.tile([128, 1], F32, name="zero128")
        nc.vector.memset(zero128, 0.0)
        wz = const.tile([128, 128], BF16, name="wz")
        nc.gpsimd.memset(wz, 0.0)

        # ---- groupnorm, pipelined per 128-channel chunk (groups are 16
        # channels, so no group spans a chunk): each chunk's scale/bias chain
        # runs as soon as its stats are in. Chunk 0's sums ride the ACT
        # accumulator (otherwise-idle engine), chunks 1-3 use DVE bn_stats.
        bnrec = small.tile([128, 4, 2, 6], F32, name="bnrec")
        mv = small.tile([128, 4, 2], F32, name="mv")
        stats = small.tile([128, 4, 2], F32, name="stats")
        acc = small.tile([128, 2], F32, name="acc")
        scr = big.tile([128, L], BF16, name="scr")
        sb2 = small.tile([8, 4, 2], F32, name="sb2")
        lnv = small.tile([8, 4], F32, name="lnv")
        gstat = small.tile([8, 4, 2], F32, name="gstat")
        mb = small.tile([128, 4, 2], F32, name="mb")
        # xhat = xb*rstd - mean*rstd in fp8 DoubleRow planes (DVE only:
        # Pool's fp8 path is a ~20x-slower software fallback)
        xh8 = big.tile([128, 2, 2, L], F8, name="xh8")
        wps = [warm_ps.tile([128, 512], F32, name=f"wps{i}") for i in range(2)]

        # chunk 0: raw sum and sum-of-squares via the ACT accumulator
        nc.scalar.activation(
            out=scr, in_=xb_sb[0], func=Act.Copy, accum_out=acc[:, 0:1]
        )
        nc.scalar.activation(
            out=scr, in_=xb_sb[0], func=Act.Square, bias=zero128,
            accum_out=acc[:, 1:2],
        )

        def emit_bn(j):
            for h in range(2):
                nc.vector.bn_stats(
                    out=bnrec[:, j, h, :], in_=xb_sb[j][:, h * 512 : (h + 1) * 512]
                )
            nc.vector.bn_aggr(out=mv[:, j, :], in_=bnrec[:, j, :, :])
            # stats_j = [mean | E[x^2]] per partition, inline on DVE
            nc.vector.tensor_copy(out=stats[:, j, 0:1], in_=mv[:, j, 0:1])
            nc.vector.tensor_mul(
                out=stats[:, j, 1:2], in0=mv[:, j, 0:1], in1=mv[:, j, 0:1]
            )
            nc.vector.tensor_add(
                out=stats[:, j, 1:2], in0=stats[:, j, 1:2], in1=mv[:, j, 1:2]
            )

        def emit_gn_chain(j):
            gps = gn_ps.tile([8, 2], F32, name="gps", tag="gps")
            if j == 0:
                nc.tensor.matmul(
                    gps, lhsT=gg_sb[:, 144:152], rhs=acc, start=True, stop=True
                )
            else:
                nc.tensor.matmul(
                    gps, lhsT=gg_sb[:, 0:8], rhs=stats[:, j, :],
                    start=True, stop=True,
                )
            nc.vector.tensor_copy(out=gstat[:, j, :], in_=gps)
            var = gstat[:, j, 1:2]
            nc.vector.tensor_mul(
                out=var, in0=gstat[:, j, 0:1], in1=gstat[:, j, 0:1]
            )
            nc.vector.tensor_sub(out=var, in0=gps[:, 1:2], in1=var)
            # rstd = exp(-0.5*ln(var+eps)); ln/exp share one ACT table set
            nc.scalar.activation(
                out=lnv[:, j : j + 1], in_=var, func=Act.Ln, bias=eps8
            )
            nc.scalar.activation(
                out=sb2[:, j, 0:1], in_=lnv[:, j : j + 1], func=Act.Exp,
                bias=zero8, scale=-0.5,
            )
            # sb2[.,1] = mean*rstd; the sign flip rides the xhat subtract
            nc.vector.tensor_mul(
                out=sb2[:, j, 1:2], in0=gstat[:, j, 0:1], in1=sb2[:, j, 0:1]
            )
            mps = gn_ps.tile([128, 2], F32, name="mps", tag="mps")
            nc.tensor.matmul(
                mps, lhsT=gg_sb[0:8, 16:144], rhs=sb2[:, j, :], start=True, stop=True
            )
            nc.vector.tensor_copy(out=mb[:, j, :], in_=mps)
            nc.vector.tensor_scalar(
                out=xh8[:, j // 2, j % 2, :],
                in0=xb_sb[j],
                scalar1=mb[:, j, 0:1],
                scalar2=mb[:, j, 1:2],
                op0=Alu.mult,
                op1=Alu.subtract,
            )

        # PE warmup matmuls push the HAM clock gate to 8/8 before the real
        # qkv stream (results never read); interleaved so PE stays busy
        # through the groupnorm chains.
        def warm(n):
            for r in range(n):
                nc.tensor.matmul(
                    wps[r % 2], lhsT=wz, rhs=xb_sb[0][:, 0:512],
                    start=True, stop=True,
                )

        emit_bn(1)
        emit_bn(2)
        warm(4)
        emit_gn_chain(0)
        emit_bn(3)
        emit_gn_chain(1)
        warm(4)
        emit_gn_chain(2)
        emit_gn_chain(3)
        warm(4)
        warm_stack.close()
        gn_stack.close()

        mid_stack = ExitStack()
        qk_ps = mid_stack.enter_context(
            tc.tile_pool(name="qk_ps", bufs=2, space="PSUM")
        )
        qkv_stack = ExitStack()
        qkv_ps = qkv_stack.enter_context(
            tc.tile_pool(name="qkv_ps", bufs=4, space="PSUM")
        )

        # bv broadcast across partitions once; each v drain then adds it
        bvt = big.tile([128, 512], BF16, name="bvt")

        def emit_bvt():
            bvt_ps = qkv_ps.tile([128, 512], F32, name="bvt_ps", tag="qkvps")
            nc.tensor.matmul(
                bvt_ps, lhsT=ones_col, rhs=bv_sb, start=True, stop=True
            )
            nc.vector.tensor_copy(out=bvt, in_=bvt_ps)

        q_sb = big.tile([128, 4, L], BF16, name="q_sb")
        k_sb = big.tile([128, 4, L], BF16, name="k_sb")
        # v8: [s-in-block, jp, ko, head, ch+ones(pad to 72)] fp8 DoubleRow lhsT
        v8 = big.tile([128, 4, 2, NH, 72], F8, name="v8")
        nc.gpsimd.memset(v8[:, :, :, :, CH : CH + 1], 1.0)
        # attention output, split per (channel-chunk, t-half) so the Tile
        # dependency tracker sees no false proj-read vs av-write conflicts
        a_t = [
            [big.tile([128, 512], BF16, name=f"a{j}t{t}") for t in range(2)]
            for j in range(4)
        ]

        def emit_qk_chunk(i):
            # q,k output chunk i (head pair), both t-halves; fp8 DoubleRow,
            # contraction (jp, ko) over the 4 channel blocks.
            for mi, (mat, bcol) in enumerate(((q_sb, i), (k_sb, 4 + i))):
                pss = [
                    qkv_ps.tile([128, 512], F32, name="qkvps", tag="qkvps")
                    for _ in range(2)
                ]
                for jp in range(2):
                    for t in range(2):
                        nc.tensor.matmul(
                            pss[t],
                            lhsT=w8_sb[:, mi, jp, :, i * 128 : (i + 1) * 128],
                            rhs=xh8[:, jp, :, t * 512 : (t + 1) * 512],
                            start=(jp == 0),
                            stop=(jp == 1),
                            perf_mode=DR,
                        )
                        if jp == 1:
                            nc.vector.tensor_scalar_add(
                                out=mat[:, i, t * 512 : (t + 1) * 512],
                                in0=pss[t],
                                scalar1=gg_sb[:, 8 + bcol : 9 + bcol],
                            )

        def emit_v():
            for li in range(8):
                ps = qkv_ps.tile([128, 512], F32, name="qkvps", tag="qkvps")
                for jp in range(2):
                    nc.tensor.matmul(
                        ps,
                        lhsT=xh8[:, jp, :, li * 128 : (li + 1) * 128],
                        rhs=w8_sb[:, 2, jp, :, :],
                        start=(jp == 0),
                        stop=(jp == 1),
                        perf_mode=DR,
                    )
                nc.vector.tensor_add(
                    out=v8[:, li // 2, li % 2, :, 0:CH],
                    in0=ps.rearrange("p (h c) -> p h c", h=NH),
                    in1=bvt.rearrange("p (h c) -> p h c", h=NH),
                )

        pt_tiles = {}

        def emit_qk_exp(pr, th, inject=None):
            # pt8: [s-in-block, h01, jp, ko, t] fp8 softmax numerators.
            # inject[j] emits extra PE work after score-pair j, filling the
            # PE idle slots between WAR-paced pairs late in the stream.
            pt = ptp.tile([128, 2, 4, 2, 512], F8, name="pt", tag="pt")
            pt_tiles[(pr, th)] = pt
            for j in range(8):
                st = qk_ps.tile([128, 2, 512], F32, name="st", tag="st")
                for h01 in range(2):
                    r0, r1 = h01 * 64, (h01 + 1) * 64
                    nc.tensor.matmul(
                        st[:, h01, :],
                        lhsT=k_sb[r0:r1, pr, j * 128 : (j + 1) * 128],
                        rhs=q_sb[r0:r1, pr, th * 512 : (th + 1) * 512],
                        start=True,
                        stop=True,
                    )
                nc.scalar.activation(
                    out=pt[:, :, j // 2, j % 2, :], in_=st, func=Act.Exp, bias=noff
                )
                if inject and j in inject:
                    inject[j]()

        def emit_av_mm(pr, th, h01, ps_pool):
            # [CH+1, 512] over the 4 jp pairs in fp8 DoubleRow (ones column
            # -> softmax denominator in row CH), drained to bf16 au.
            pt = pt_tiles[(pr, th)]
            sl = slice(th * 512, (th + 1) * 512)
            h = 2 * pr + h01
            av = ps_pool.tile([CH + 1, 512], F32, name="av", tag="av")
            for jp in range(4):
                nc.tensor.matmul(
                    av,
                    lhsT=v8[:, jp, :, h, 0 : CH + 1],
                    rhs=pt[:, h01, jp, :, :],
                    start=(jp == 0),
                    stop=(jp == 3),
                    perf_mode=DR,
                )
            au = aup.tile([CH + 1, 512], BF16, name="au", tag="au")
            nc.vector.tensor_copy(out=au, in_=av)
            return au

        def emit_av_act_div(pr, th, aus):
            # tail unit: pipelined ACT divide (1/d = exp(-ln d)) and K=1
            # ones-matmul partition-broadcasts; AV matmuls done by caller.
            sl = slice(th * 512, (th + 1) * 512)
            rrows = []
            for h01 in range(2):
                lrow = tailp.tile([1, 512], F32, name="lrow", tag="lrow")
                rrowb = tailp.tile([1, 512], BF16, name="rrowb", tag=f"rrowb{h01}")
                nc.scalar.activation(
                    out=lrow, in_=aus[h01][CH : CH + 1, :], func=Act.Ln, bias=zero1
                )
                nc.scalar.activation(
                    out=rrowb, in_=lrow, func=Act.Exp, bias=zero1, scale=-1.0
                )
                rrows.append(rrowb)
            for h01 in range(2):
                ro = h01 * 64
                bc = bc_ps.tile([CH, 512], F32, name="bc", tag="bc")
                nc.tensor.matmul(
                    bc, lhsT=ones64, rhs=rrows[h01], start=True, stop=True
                )
                nc.vector.tensor_mul(
                    out=a_t[pr][th][ro : ro + 64, :], in0=aus[h01][0:CH, :], in1=bc
                )

        av_state = {}

        def emit_av_h(pr, th, h01, ps_pool):
            # one head's AV matmuls + drain + denominator row to DRAM
            sl = slice(th * 512, (th + 1) * 512)
            h = 2 * pr + h01
            au = emit_av_mm(pr, th, h01, ps_pool)
            nc.sync.dma_start(out=rdram[h : h + 1, sl], in_=au[CH : CH + 1, :])
            av_state[(pr, th, h01)] = au

        def emit_av_fin(pr, th):
            # packed reciprocal + broadcast + normalize for both heads
            sl = slice(th * 512, (th + 1) * 512)
            au_tiles = {h01: av_state.pop((pr, th, h01)) for h01 in range(2)}
            pt_tiles.pop((pr, th))
            # packed reciprocal for both heads' denominator rows via DRAM
            # bounce ([2,512] -> [32,32] -> recip -> bf16 -> back).
            lpk = pkp.tile([32, 32], BF16, name="lpk", tag="lpk")
            b1 = rdram[2 * pr : 2 * pr + 2, sl]
            nc.sync.dma_start(
                out=lpk,
                in_=bass.AP(
                    tensor=b1.tensor, offset=b1.offset,
                    ap=[[L, 2], [32, 16], [1, 32]],
                ),
            )
            rpk = pkp.tile([32, 32], F32, name="rpk", tag="rpk")
            nc.vector.reciprocal(out=rpk, in_=lpk)
            rpkb = pkp.tile([32, 32], BF16, name="rpkb", tag="rpkb")
            nc.gpsimd.tensor_copy(out=rpkb, in_=rpk)
            b2 = rdram2[2 * pr : 2 * pr + 2, sl]
            nc.sync.dma_start(
                out=bass.AP(
                    tensor=b2.tensor, offset=b2.offset,
                    ap=[[L, 2], [32, 16], [1, 32]],
                ),
                in_=rpkb,
            )
            for h01 in range(2):
                h = 2 * pr + h01
                ro = h01 * 64
                rbc = rbcp.tile([CH, 512], BF16, name="rbc", tag="rbc")
                nc.sync.dma_start(
                    out=rbc,
                    in_=rdram2[h : h + 1, sl].partition_broadcast(CH),
                )
                nc.vector.tensor_mul(
                    out=a_t[pr][th][ro : ro + 64, :],
                    in0=au_tiles.pop(h01)[0:CH, :],
                    in1=rbc,
                )

        # ---- middle: th-major unit order (0,0),(1,0),(2,0),(3,0),(0,1)...
        # so the whole t=0 half (attention, divide chains, projection and
        # half the output DMA) completes mid-stream; av(u) work is injected
        # into the exp emitter ~2 units later, filling the PE idle slots
        # between WAR-paced score pairs without ever gating the exp stream.
        emit_qk_chunk(0)
        emit_qk_exp(0, 0)
        emit_bvt()
        emit_qk_chunk(1)
        emit_qk_exp(1, 0)
        emit_qk_chunk(2)
        emit_qk_exp(2, 0)
        emit_qk_chunk(3)
        emit_v()
        # f32 residual x: only needed at the epilogue; rides the gpsimd ring
        # so the sync ring stays clear for the reciprocal bounces.
        x_sb = []
        for jj in range(2):
            xt = big.tile([128, 2, L], F32, name=f"x{jj}")
            nc.gpsimd.dma_start(out=xt, in_=x_p[:, 2 * jj : 2 * jj + 2, :])
            x_sb.append(xt)
        qkv_stack.close()
        proj_stack = ExitStack()
        proj_ps = proj_stack.enter_context(
            tc.tile_pool(name="proj_ps", bufs=2, space="PSUM")
        )
        av_stack = ExitStack()
        av_ps = av_stack.enter_context(
            tc.tile_pool(name="av_ps", bufs=2, space="PSUM")
        )

        def emit_proj(t, i_pair, js, start, stop, pps=None):
            # proj for output chunks i_pair over contraction blocks js
            sl = slice(t * 512, (t + 1) * 512)
            if pps is None:
                pps = [
                    proj_ps.tile([128, 512], F32, name="pps", tag="pps")
                    for _ in i_pair
                ]
            for j in js:
                for n, i in enumerate(i_pair):
                    nc.tensor.matmul(
                        pps[n],
                        lhsT=pw_sb[:, j, i * 128 : (i + 1) * 128],
                        rhs=a_t[j][t],
                        start=(start and j == js[0]),
                        stop=(stop and j == js[-1]),
                    )
            return pps

        def emit_proj_out(t, i_pair, pps, eng):
            # residual add + batched output DMA for chunk pair i_pair
            sl = slice(t * 512, (t + 1) * 512)
            ott = outp.tile([128, 2, 512], F32, name="ot", tag="ot")
            for n, i in enumerate(i_pair):
                nc.vector.tensor_add(
                    out=ott[:, n, :], in0=pps[n], in1=x_sb[i // 2][:, i % 2, sl]
                )
            eng.dma_start(
                out=out_p[:, i_pair[0] : i_pair[0] + 2, sl], in_=ott
            )

        def emit_av_full(pr, th):
            emit_av_h(pr, th, 0, av_ps)
            emit_av_h(pr, th, 1, av_ps)
            emit_av_fin(pr, th)

        emit_av_full(0, 0)
        emit_qk_exp(3, 0)
        emit_av_full(1, 0)
        emit_qk_exp(0, 1)
        emit_av_full(2, 0)
        emit_qk_exp(1, 1)
        emit_av_full(3, 0)
        emit_qk_exp(2, 1)
        emit_av_full(0, 1)
        emit_qk_exp(3, 1)
        emit_av_full(1, 1)
        # t=0 projection: all inputs completed mid-stream; with per-half a
        # tiles there is no false WAR against the t=1 AV writes, so it can
        # fill the PE idle while av(2,1) waits on the tail exps.
        pp0a = emit_proj(0, (0, 1), [0, 1, 2, 3], True, True)
        emit_proj_out(0, (0, 1), pp0a, nc.gpsimd)
        emit_av_full(2, 1)
        pp0b = emit_proj(0, (2, 3), [0, 1, 2, 3], True, True)
        emit_proj_out(0, (2, 3), pp0b, nc.gpsimd)
        av_stack.close()
        bc_stack = ExitStack()
        av2_ps = bc_stack.enter_context(
            tc.tile_pool(name="av2_ps", bufs=1, space="PSUM")
        )
        bc_ps = bc_stack.enter_context(
            tc.tile_pool(name="bc_ps", bufs=1, space="PSUM")
        )
        # tail: last unit's AV matmuls, then t=1 partials keep the PE warm
        # under the ACT divide chain, then the j-ordered t=1 projection.
        pp1a = emit_proj(1, (0, 1), [0, 1], True, False)
        aus31 = [emit_av_mm(3, 1, h01, av2_ps) for h01 in range(2)]
        pt_tiles.pop((3, 1))
        emit_proj(1, (0, 1), [2], False, False, pps=pp1a)
        emit_av_act_div(3, 1, aus31)
        emit_proj(1, (0, 1), [3], False, True, pps=pp1a)
        emit_proj_out(1, (0, 1), pp1a, nc.scalar)
        pp1b = emit_proj(1, (2, 3), [0, 1, 2, 3], True, True)
        emit_proj_out(1, (2, 3), pp1b, nc.scalar)
        bc_stack.close()
        proj_stack.close()
        mid_stack.close()

    _split_excess_waits(nc, mybir)
    return nc


def _split_excess_waits(nc, mybir, max_waits=1):
    """This toolchain's walrus rejects engine instructions carrying more
    than one sync-wait command; hoist extras onto NoOps placed just before
    (same engine, so ordering is preserved)."""
    n_split = 0
    for fn in nc.m.functions:
        for bb in fn.blocks:
            out = []
            for inst in bb.instructions:
                si = inst.sync_info
                if si is not None and si.on_wait and len(si.on_wait) > max_waits:
                    waits = list(si.on_wait)
                    hoist, keep = waits[:-max_waits], waits[-max_waits:]
                    for k, w in enumerate(hoist):
                        nop = mybir.InstNoOp(
                            name=f"{inst.name}_hw{k}", ins=[], outs=[],
                            engine=inst.engine,
                        )
                        nop.sync_info = mybir.SyncInfo(on_wait=[w], on_update=[])
                        out.append(nop)
                    inst.sync_info = mybir.SyncInfo(
                        on_wait=keep, on_update=list(si.on_update or [])
                    )
                    n_split += 1
                out.append(inst)
            bb.instructions = out
    return n_split


def _prep_weights(gn_w, gn_b, qkv_w, qkv_b, proj_w):
    bf16 = ml_dtypes.bfloat16
    f8 = ml_dtypes.float8_e4m3
    scale = 1.0 / math.sqrt(math.sqrt(CH))
    w_eff = (qkv_w.astype(np.float64) * gn_w[None, :].astype(np.float64))
    b_eff = qkv_b.astype(np.float64) + qkv_w.astype(np.float64) @ gn_b.astype(
        np.float64
    )
    o = np.arange(3 * C)
    within = o % (3 * CH)
    rowscale = np.where(within < 2 * CH, scale, 1.0)
    w_eff = (w_eff * rowscale[:, None]).astype(np.float32)
    b_eff = (b_eff * rowscale).astype(np.float32)
    heads = np.arange(NH)[:, None] * 3 * CH
    perm_q = (heads + np.arange(CH)[None, :]).ravel()
    perm_k = (heads + CH + np.arange(CH)[None, :]).ravel()
    perm_v = (heads + 2 * CH + np.arange(CH)[None, :]).ravel()

    def dev_w8(w):  # [O, C] -> DoubleRow lhsT [128, jp, ko, O]
        # contraction channel (2*jp+ko)*128 + p
        return np.ascontiguousarray(
            w.T.reshape(2, 2, 128, w.shape[0]).transpose(2, 0, 1, 3)
        ).astype(f8)

    def dev_w(w):  # [O, C] -> bf16 lhsT chunks [128, 4, O]
        return np.ascontiguousarray(
            w.T.reshape(4, 128, w.shape[0]).transpose(1, 0, 2)
        ).astype(bf16)

    def dev_b(b):  # [512] -> [128, 4]
        return np.ascontiguousarray(b.reshape(4, 128).T).astype(np.float32)

    w8 = np.stack(
        [dev_w8(w_eff[perm_q]), dev_w8(w_eff[perm_k]), dev_w8(w_eff[perm_v])],
        axis=1,
    )
    bqk = np.concatenate([dev_b(b_eff[perm_q]), dev_b(b_eff[perm_k])], axis=1)
    g = (np.arange(128)[:, None] // 16 == np.arange(8)[None, :]).astype(np.float32)
    gg = np.zeros((128, 160), np.float32)
    gg[:, 0:8] = g / 16.0  # group-size divisor folded into the reduce matmul
    gg[:, 8:16] = bqk
    gg[0:8, 16:144] = g.T
    gg[:, 144:152] = g / (16.0 * L)  # for the ACT raw-sum stats path
    return {
        "w8": np.ascontiguousarray(w8),
        "pw": dev_w(proj_w.astype(np.float32)),
        "bv": b_eff[perm_v].reshape(1, C).astype(bf16),
        "gg": gg,
    }


def run(inputs, trace=False):
    import time

    from concourse.bass_utils import run_bass_kernel_spmd

    t0 = time.time()
    if "nc" not in _cache:
        _cache["nc"] = _build_program()
    nc = _cache["nc"]
    print(f"[kernel] program built in {time.time() - t0:.1f}s", flush=True)

    x = np.asarray(inputs["x"], dtype=np.float32)
    proj_b = np.asarray(inputs["proj_b"], dtype=np.float32)
    shared = _prep_weights(
        np.asarray(inputs["gn_w"], dtype=np.float32),
        np.asarray(inputs["gn_b"], dtype=np.float32),
        np.asarray(inputs["qkv_w"], dtype=np.float32),
        np.asarray(inputs["qkv_b"], dtype=np.float32),
        np.asarray(inputs["proj_w"], dtype=np.float32),
    )
    in_maps = []
    for b in range(NCORES):
        xb = np.ascontiguousarray(x[b].reshape(C, L))
        # residual + proj bias folded on host: out = (x + proj_b) + pw @ a
        xpb = (xb.astype(np.float64) + proj_b[:, None].astype(np.float64)).astype(
            np.float32
        )
        in_maps.append(
            {**shared, "x": xpb, "xb": xb.astype(ml_dtypes.bfloat16)}
        )
    t1 = time.time()
    res = run_bass_kernel_spmd(
        nc, in_maps, list(range(NCORES)), trace=trace,
        tmpdir=os.environ.get("BASS_KERNEL_TMPDIR"),
    )
    print(f"[kernel] executed in {time.time() - t1:.1f}s", flush=True)
    out = np.stack([res.results[b]["out"] for b in range(NCORES)])
    return out.reshape(B, C, HH, WW), res


def kernel(**inputs):
    out, _ = run(inputs)
    return out


# revision 65
# speedup vs baseline: 1.0145x; 1.0145x over previous
"""AttentionBlock (GroupNorm + 1x1-conv qkv + MHA + proj + residual) on 8 trn2 cores.

Sharding: data-parallel over batch (B=8 -> 1 batch item per core); weights
replicated. Each core computes its full attention block on x[b] in [C, L]
layout (C=512 channels, L=1024 positions).

v2 design (vs the bf16 baseline):
  - qkv and AV matmuls run in fp8(e4m3) DoubleRow mode (2 fp8 weights/cell,
    contraction 256 per matmul) -> ~2x PE throughput on those stages. QK and
    proj stay bf16 (QK pairs already run 2x via row-tiling; proj keeps the
    j-split tail cheap and accurate).
  - softmax exp emits fp8 P directly: exp(s - 3) (offset keeps max ~30 << 240
    e4m3 limit; offset cancels between numerator and ones-column denominator).
  - groupnorm stats all on DVE (bn_stats), so ACT only ever loads one table
    set and the exp stream can start ~10us in (vs ~29us).
  - proj t-half epilogues batched into single 1MB output DMAs (one per half)
    on otherwise-idle engine rings (gpsimd / scalar) to kill the DMA drain
    tail.
  - last unit's softmax-denominator reciprocal runs on ACT (ln/exp) + a K=1
    ones-matmul partition-broadcast, skipping two DRAM round trips on the
    critical tail; earlier units keep the packed-DVE-reciprocal DRAM bounce
    (fully hidden mid-stream).

Host-side preprocessing (exact math, no approximation beyond dtype casts):
  - GroupNorm affine (gn_w, gn_b) folded into qkv_w/qkv_b.
  - Attention scale (1/sqrt(sqrt(ch)) on both q and k) folded into qkv rows.
  - qkv rows permuted to head-major [q_heads | k_heads | v_heads].
  - proj bias folded into the f32 residual copy of x (out = (x+proj_b) + pw@a).
  - Weights pre-transposed into the PE's stationary (lhsT = [K, M]) layout;
    fp8 weights pre-paired for DoubleRow ([K, jp, ko, M] with contraction
    channel (2*jp+ko)*128 + K).
"""

import math
import os
from contextlib import ExitStack

import ml_dtypes
import numpy as np

B, C, HH, WW = 8, 512, 32, 32
L = HH * WW          # 1024
NH = 8               # heads
NG = 32              # groupnorm groups
CH = C // NH         # 64 head dim
EPS = 1e-5
NCORES = 8
OFF = 3.0            # exp offset: p = exp(s - OFF), max ~e^3.4 ~ 30 << 240

_cache = {}


def _build_program():
    import concourse.bass as bass
    import concourse.mybir as mybir
    import concourse.tile as tile

    F32 = mybir.dt.float32
    BF16 = mybir.dt.bfloat16
    F8 = mybir.dt.float8e4
    Alu = mybir.AluOpType
    Act = mybir.ActivationFunctionType
    DR = mybir.MatmulPerfMode.DoubleRow

    nc = bass.Bass()

    xb_d = nc.dram_tensor("xb", [C, L], BF16, kind="ExternalInput")
    x_d = nc.dram_tensor("x", [C, L], F32, kind="ExternalInput")
    # fp8 DoubleRow lhsT for q,k,v: [p, which(3), jp(2), ko(2), out(512)]
    w8_d = nc.dram_tensor("w8", [128, 3, 2, 2, C], F8, kind="ExternalInput")
    pw_d = nc.dram_tensor("pw", [128, 4, C], BF16, kind="ExternalInput")
    bv_d = nc.dram_tensor("bv", [1, C], BF16, kind="ExternalInput")
    # packed small constants: cols 0:8 g/16, 8:16 [bq|bk], rows 0:8 of
    # cols 16:144 hold gt, cols 144:152 g/(16*L) (for the ACT sum-stats path)
    gg_d = nc.dram_tensor("gg", [128, 160], F32, kind="ExternalInput")
    out_d = nc.dram_tensor("out", [C, L], F32, kind="ExternalOutput")
    # DRAM bounce buffers for the softmax denominators (units 0..6): rdram
    # collects the raw per-(head, t) sums, rdram2 the bf16 reciprocals packed
    # for the partition-broadcast reload.
    rdram = nc.dram_tensor("rdram", [NH, L], BF16)
    rdram2 = nc.dram_tensor("rdram2", [NH, L], BF16)

    xb_p = xb_d.rearrange("(n p) l -> p n l", p=128)
    x_p = x_d.rearrange("(n p) l -> p n l", p=128)
    out_p = out_d.rearrange("(n p) l -> p n l", p=128)

    with tile.TileContext(nc) as tc, ExitStack() as stack:
        const = stack.enter_context(tc.tile_pool(name="const", bufs=1))
        big = stack.enter_context(tc.tile_pool(name="big", bufs=1))
        small = stack.enter_context(tc.tile_pool(name="small", bufs=1))
        ptp = stack.enter_context(tc.tile_pool(name="ptp", bufs=3))
        aup = stack.enter_context(tc.tile_pool(name="aup", bufs=4))
        outp = stack.enter_context(tc.tile_pool(name="outp", bufs=3))
        rbcp = stack.enter_context(tc.tile_pool(name="rbcp", bufs=4))
        pkp = stack.enter_context(tc.tile_pool(name="pkp", bufs=4))
        tailp = stack.enter_context(tc.tile_pool(name="tailp", bufs=2))
        gn_stack = ExitStack()
        gn_ps = gn_stack.enter_context(tc.tile_pool(name="gn_ps", bufs=1, space="PSUM"))
        warm_stack = ExitStack()
        warm_ps = warm_stack.enter_context(
            tc.tile_pool(name="warm_ps", bufs=1, space="PSUM")
        )

        # ---- loads. sync ring carries only the startup-critical tensors in
        # need-order (xb -> stats; gg -> group reduce; w8 -> qkv matmuls);
        # everything else rides the gpsimd SWDGE ring so the sync triggers
        # (~0.7us each, serialized) don't push them out.
        xb_sb = []
        for jj in range(2):
            xbt = big.tile([128, 2, L], BF16, name=f"xb{jj}")
            nc.sync.dma_start(out=xbt, in_=xb_p[:, 2 * jj : 2 * jj + 2, :])
            xb_sb.append(xbt[:, 0, :])
            xb_sb.append(xbt[:, 1, :])
        gg_sb = const.tile([128, 160], F32, name="gg_sb")
        nc.sync.dma_start(out=gg_sb, in_=gg_d[:])
        w8_sb = const.tile([128, 3, 2, 2, C], F8, name="w8_sb")
        nc.sync.dma_start(out=w8_sb, in_=w8_d[:])
        bv_sb = const.tile([1, C], BF16, name="bv_sb")
        nc.gpsimd.dma_start(out=bv_sb, in_=bv_d[:])
        pw_sb = const.tile([128, 4, C], BF16, name="pw_sb")
        nc.gpsimd.dma_start(out=pw_sb, in_=pw_d[:])

        ones_col = const.tile([1, 128], BF16, name="ones_col")
        nc.vector.memset(ones_col, 1.0)
        ones64 = const.tile([1, CH], BF16, name="ones64")
        nc.vector.memset(ones64, 1.0)
        eps8 = const.tile([8, 1], F32, name="eps8")
        nc.vector.memset(eps8, EPS)
        zero8 = const.tile([8, 1], F32, name="zero8")
        nc.vector.memset(zero8, 0.0)
        zero1 = const.tile([1, 1], F32, name="zero1")
        nc.vector.memset(zero1, 0.0)
        noff = const.tile([128, 1], F32, name="noff")
        nc.vector.memset(noff, -OFF)
        zero128 = const.tile([128, 1], F32, name="zero128")
        nc.vector.memset(zero128, 0.0)
        wz = const.tile([128, 128], BF16, name="wz")
        nc.gpsimd.memset(wz, 0.0)

        # ---- groupnorm, pipelined per 128-channel chunk (groups are 16
        # channels, so no group spans a chunk): each chunk's scale/bias chain
        # runs as soon as its stats are in. Chunk 0's sums ride the ACT
        # accumulator (otherwise-idle engine), chunks 1-3 use DVE bn_stats.
        bnrec = small.tile([128, 4, 2, 6], F32, name="bnrec")
        mv = small.tile([128, 4, 2], F32, name="mv")
        stats = small.tile([128, 4, 2], F32, name="stats")
        acc = small.tile([128, 2], F32, name="acc")
        scr = big.tile([128, L], BF16, name="scr")
        sb2 = small.tile([8, 4, 2], F32, name="sb2")
        lnv = small.tile([8, 4], F32, name="lnv")
        gstat = small.tile([8, 4, 2], F32, name="gstat")
        mb = small.tile([128, 4, 2], F32, name="mb")
        # xhat = xb*rstd - mean*rstd in fp8 DoubleRow planes (DVE only:
        # Pool's fp8 path is a ~20x-slower software fallback)
        xh8 = big.tile([128, 2, 2, L], F8, name="xh8")
        wps = [warm_ps.tile([128, 512], F32, name=f"wps{i}") for i in range(2)]

        # chunk 0: raw sum and sum-of-squares via the ACT accumulator
        nc.scalar.activation(
            out=scr, in_=xb_sb[0], func=Act.Copy, accum_out=acc[:, 0:1]
        )
        nc.scalar.activation(
            out=scr, in_=xb_sb[0], func=Act.Square, bias=zero128,
            accum_out=acc[:, 1:2],
        )

        def emit_bn(j):
            for h in range(2):
                nc.vector.bn_stats(
                    out=bnrec[:, j, h, :], in_=xb_sb[j][:, h * 512 : (h + 1) * 512]
                )
            nc.vector.bn_aggr(out=mv[:, j, :], in_=bnrec[:, j, :, :])
            # stats_j = [mean | E[x^2]] per partition, inline on DVE
            nc.vector.tensor_copy(out=stats[:, j, 0:1], in_=mv[:, j, 0:1])
            nc.vector.tensor_mul(
                out=stats[:, j, 1:2], in0=mv[:, j, 0:1], in1=mv[:, j, 0:1]
            )
            nc.vector.tensor_add(
                out=stats[:, j, 1:2], in0=stats[:, j, 1:2], in1=mv[:, j, 1:2]
            )

        def emit_gn_chain(j):
            gps = gn_ps.tile([8, 2], F32, name="gps", tag="gps")
            if j == 0:
                nc.tensor.matmul(
                    gps, lhsT=gg_sb[:, 144:152], rhs=acc, start=True, stop=True
                )
            else:
                nc.tensor.matmul(
                    gps, lhsT=gg_sb[:, 0:8], rhs=stats[:, j, :],
                    start=True, stop=True,
                )
            nc.vector.tensor_copy(out=gstat[:, j, :], in_=gps)
            var = gstat[:, j, 1:2]
            nc.vector.tensor_mul(
                out=var, in0=gstat[:, j, 0:1], in1=gstat[:, j, 0:1]
            )
            nc.vector.tensor_sub(out=var, in0=gps[:, 1:2], in1=var)
            # rstd = exp(-0.5*ln(var+eps)); ln/exp share one ACT table set
            nc.scalar.activation(
                out=lnv[:, j : j + 1], in_=var, func=Act.Ln, bias=eps8
            )
            nc.scalar.activation(
                out=sb2[:, j, 0:1], in_=lnv[:, j : j + 1], func=Act.Exp,
                bias=zero8, scale=-0.5,
            )
            # sb2[.,1] = mean*rstd; the sign flip rides the xhat subtract
            nc.vector.tensor_mul(
                out=sb2[:, j, 1:2], in0=gstat[:, j, 0:1], in1=sb2[:, j, 0:1]
            )
            mps = gn_ps.tile([128, 2], F32, name="mps", tag="mps")
            nc.tensor.matmul(
                mps, lhsT=gg_sb[0:8, 16:144], rhs=sb2[:, j, :], start=True, stop=True
            )
            nc.vector.tensor_copy(out=mb[:, j, :], in_=mps)
            nc.vector.tensor_scalar(
                out=xh8[:, j // 2, j % 2, :],
                in0=xb_sb[j],
                scalar1=mb[:, j, 0:1],
                scalar2=mb[:, j, 1:2],
                op0=Alu.mult,
                op1=Alu.subtract,
            )

        # PE warmup matmuls push the HAM clock gate to 8/8 before the real
        # qkv stream (results never read); interleaved so PE stays busy
        # through the groupnorm chains.
        def warm(n):
            for r in range(n):
                nc.tensor.matmul(
                    wps[r % 2], lhsT=wz, rhs=xb_sb[0][:, 0:512],
                    start=True, stop=True,
                )

        emit_bn(1)
        emit_bn(2)
        warm(4)
        emit_gn_chain(0)
        emit_bn(3)
        emit_gn_chain(1)
        warm(4)
        emit_gn_chain(2)
        emit_gn_chain(3)
        warm(4)
        warm_stack.close()
        gn_stack.close()

        mid_stack = ExitStack()
        qk_ps = mid_stack.enter_context(
            tc.tile_pool(name="qk_ps", bufs=2, space="PSUM")
        )
        qkv_stack = ExitStack()
        qkv_ps = qkv_stack.enter_context(
            tc.tile_pool(name="qkv_ps", bufs=4, space="PSUM")
        )

        # bv broadcast across partitions once; each v drain then adds it
        bvt = big.tile([128, 512], BF16, name="bvt")

        def emit_bvt():
            bvt_ps = qkv_ps.tile([128, 512], F32, name="bvt_ps", tag="qkvps")
            nc.tensor.matmul(
                bvt_ps, lhsT=ones_col, rhs=bv_sb, start=True, stop=True
            )
            nc.vector.tensor_copy(out=bvt, in_=bvt_ps)

        q_sb = big.tile([128, 4, L], BF16, name="q_sb")
        k_sb = big.tile([128, 4, L], BF16, name="k_sb")
        # v8: [s-in-block, jp, ko, head, ch+ones(pad to 72)] fp8 DoubleRow lhsT
        v8 = big.tile([128, 4, 2, NH, 72], F8, name="v8")
        nc.gpsimd.memset(v8[:, :, :, :, CH : CH + 1], 1.0)
        # attention output, split per (channel-chunk, t-half) so the Tile
        # dependency tracker sees no false proj-read vs av-write conflicts
        a_t = [
            [big.tile([128, 512], BF16, name=f"a{j}t{t}") for t in range(2)]
            for j in range(4)
        ]

        def emit_qk_chunk(i):
            # q,k output chunk i (head pair), both t-halves; fp8 DoubleRow,
            # contraction (jp, ko) over the 4 channel blocks.
            for mi, (mat, bcol) in enumerate(((q_sb, i), (k_sb, 4 + i))):
                pss = [
                    qkv_ps.tile([128, 512], F32, name="qkvps", tag="qkvps")
                    for _ in range(2)
                ]
                for jp in range(2):
                    for t in range(2):
                        nc.tensor.matmul(
                            pss[t],
                            lhsT=w8_sb[:, mi, jp, :, i * 128 : (i + 1) * 128],
                            rhs=xh8[:, jp, :, t * 512 : (t + 1) * 512],
                            start=(jp == 0),
                            stop=(jp == 1),
                            perf_mode=DR,
                        )
                        if jp == 1:
                            nc.vector.tensor_scalar_add(
                                out=mat[:, i, t * 512 : (t + 1) * 512],
                                in0=pss[t],
                                scalar1=gg_sb[:, 8 + bcol : 9 + bcol],
                            )

        def emit_v():
            for li in range(8):
                ps = qkv_ps.tile([128, 512], F32, name="qkvps", tag="qkvps")
                for jp in range(2):
                    nc.tensor.matmul(
                        ps,
                        lhsT=xh8[:, jp, :, li * 128 : (li + 1) * 128],
                        rhs=w8_sb[:, 2, jp, :, :],
                        start=(jp == 0),
                        stop=(jp == 1),
                        perf_mode=DR,
                    )
                nc.vector.tensor_add(
                    out=v8[:, li // 2, li % 2, :, 0:CH],
                    in0=ps.rearrange("p (h c) -> p h c", h=NH),
                    in1=bvt.rearrange("p (h c) -> p h c", h=NH),
                )

        pt_tiles = {}

        def emit_qk_exp(pr, th, inject=None):
            # pt8: [s-in-block, h01, jp, ko, t] fp8 softmax numerators.
            # inject[j] emits extra PE work after score-pair j, filling the
            # PE idle slots between WAR-paced pairs late in the stream.
            pt = ptp.tile([128, 2, 4, 2, 512], F8, name="pt", tag="pt")
            pt_tiles[(pr, th)] = pt
            for j in range(8):
                st = qk_ps.tile([128, 2, 512], F32, name="st", tag="st")
                for h01 in range(2):
                    r0, r1 = h01 * 64, (h01 + 1) * 64
                    nc.tensor.matmul(
                        st[:, h01, :],
                        lhsT=k_sb[r0:r1, pr, j * 128 : (j + 1) * 128],
                        rhs=q_sb[r0:r1, pr, th * 512 : (th + 1) * 512],
                        start=True,
                        stop=True,
                    )
                nc.scalar.activation(
                    out=pt[:, :, j // 2, j % 2, :], in_=st, func=Act.Exp, bias=noff
                )
                if inject and j in inject:
                    inject[j]()

        def emit_av_mm(pr, th, h01, ps_pool):
            # [CH+1, 512] over the 4 jp pairs in fp8 DoubleRow (ones column
            # -> softmax denominator in row CH), drained to bf16 au.
            pt = pt_tiles[(pr, th)]
            sl = slice(th * 512, (th + 1) * 512)
            h = 2 * pr + h01
            av = ps_pool.tile([CH + 1, 512], F32, name="av", tag="av")
            for jp in range(4):
                nc.tensor.matmul(
                    av,
                    lhsT=v8[:, jp, :, h, 0 : CH + 1],
                    rhs=pt[:, h01, jp, :, :],
                    start=(jp == 0),
                    stop=(jp == 3),
                    perf_mode=DR,
                )
            au = aup.tile([CH + 1, 512], BF16, name="au", tag="au")
            nc.vector.tensor_copy(out=au, in_=av)
            return au

        def emit_av_act_div(pr, th, aus):
            # tail unit: pipelined ACT divide (1/d = exp(-ln d)) and K=1
            # ones-matmul partition-broadcasts; AV matmuls done by caller.
            sl = slice(th * 512, (th + 1) * 512)
            rrows = []
            for h01 in range(2):
                lrow = tailp.tile([1, 512], F32, name="lrow", tag="lrow")
                rrowb = tailp.tile([1, 512], BF16, name="rrowb", tag=f"rrowb{h01}")
                nc.scalar.activation(
                    out=lrow, in_=aus[h01][CH : CH + 1, :], func=Act.Ln, bias=zero1
                )
                nc.scalar.activation(
                    out=rrowb, in_=lrow, func=Act.Exp, bias=zero1, scale=-1.0
                )
                rrows.append(rrowb)
            for h01 in range(2):
                ro = h01 * 64
                bc = bc_ps.tile([CH, 512], F32, name="bc", tag="bc")
                nc.tensor.matmul(
                    bc, lhsT=ones64, rhs=rrows[h01], start=True, stop=True
                )
                nc.vector.tensor_mul(
                    out=a_t[pr][th][ro : ro + 64, :], in0=aus[h01][0:CH, :], in1=bc
                )

        av_state = {}

        def emit_av_h(pr, th, h01, ps_pool):
            # one head's AV matmuls + drain + denominator row to DRAM
            sl = slice(th * 512, (th + 1) * 512)
            h = 2 * pr + h01
            au = emit_av_mm(pr, th, h01, ps_pool)
            nc.sync.dma_start(out=rdram[h : h + 1, sl], in_=au[CH : CH + 1, :])
            av_state[(pr, th, h01)] = au

        def emit_av_fin(pr, th):
            # packed reciprocal + broadcast + normalize for both heads
            sl = slice(th * 512, (th + 1) * 512)
            au_tiles = {h01: av_state.pop((pr, th, h01)) for h01 in range(2)}
            pt_tiles.pop((pr, th))
            # packed reciprocal for both heads' denominator rows via DRAM
            # bounce ([2,512] -> [32,32] -> recip -> bf16 -> back).
            lpk = pkp.tile([32, 32], BF16, name="lpk", tag="lpk")
            b1 = rdram[2 * pr : 2 * pr + 2, sl]
            nc.sync.dma_start(
                out=lpk,
                in_=bass.AP(
                    tensor=b1.tensor, offset=b1.offset,
                    ap=[[L, 2], [32, 16], [1, 32]],
                ),
            )
            rpk = pkp.tile([32, 32], F32, name="rpk", tag="rpk")
            nc.vector.reciprocal(out=rpk, in_=lpk)
            rpkb = pkp.tile([32, 32], BF16, name="rpkb", tag="rpkb")
            nc.gpsimd.tensor_copy(out=rpkb, in_=rpk)
            b2 = rdram2[2 * pr : 2 * pr + 2, sl]
            nc.sync.dma_start(
                out=bass.AP(
                    tensor=b2.tensor, offset=b2.offset,
                    ap=[[L, 2], [32, 16], [1, 32]],
                ),
                in_=rpkb,
            )
            for h01 in range(2):
                h = 2 * pr + h01
                ro = h01 * 64
                rbc = rbcp.tile([CH, 512], BF16, name="rbc", tag="rbc")
                nc.sync.dma_start(
                    out=rbc,
                    in_=rdram2[h : h + 1, sl].partition_broadcast(CH),
                )
                nc.vector.tensor_mul(
                    out=a_t[pr][th][ro : ro + 64, :],
                    in0=au_tiles.pop(h01)[0:CH, :],
                    in1=rbc,
                )

        # ---- middle: th-major unit order (0,0),(1,0),(2,0),(3,0),(0,1)...
        # so the whole t=0 half (attention, divide chains, projection and
        # half the output DMA) completes mid-stream; av(u) work is injected
        # into the exp emitter ~2 units later, filling the PE idle slots
        # between WAR-paced score pairs without ever gating the exp stream.
        emit_qk_chunk(0)
        emit_qk_exp(0, 0)
        emit_bvt()
        emit_qk_chunk(1)
        emit_qk_exp(1, 0)
        emit_qk_chunk(2)
        emit_qk_exp(2, 0)
        emit_qk_chunk(3)
        emit_v()
        # f32 residual x: only needed at the epilogue; rides the gpsimd ring
        # so the sync ring stays clear for the reciprocal bounces.
        x_sb = []
        for jj in range(2):
            xt = big.tile([128, 2, L], F32, name=f"x{jj}")
            nc.gpsimd.dma_start(out=xt, in_=x_p[:, 2 * jj : 2 * jj + 2, :])
            x_sb.append(xt)
        qkv_stack.close()
        proj_stack = ExitStack()
        proj_ps = proj_stack.enter_context(
            tc.tile_pool(name="proj_ps", bufs=2, space="PSUM")
        )
        av_stack = ExitStack()
        av_ps = av_stack.enter_context(
            tc.tile_pool(name="av_ps", bufs=2, space="PSUM")
        )

        def emit_proj(t, i_pair, js, start, stop, pps=None):
            # proj for output chunks i_pair over contraction blocks js
            sl = slice(t * 512, (t + 1) * 512)
            if pps is None:
                pps = [
                    proj_ps.tile([128, 512], F32, name="pps", tag="pps")
                    for _ in i_pair
                ]
            for j in js:
                for n, i in enumerate(i_pair):
                    nc.tensor.matmul(
                        pps[n],
                        lhsT=pw_sb[:, j, i * 128 : (i + 1) * 128],
                        rhs=a_t[j][t],
                        start=(start and j == js[0]),
                        stop=(stop and j == js[-1]),
                    )
            return pps

        def emit_proj_out(t, i_pair, pps, eng):
            # residual add + batched output DMA for chunk pair i_pair
            sl = slice(t * 512, (t + 1) * 512)
            ott = outp.tile([128, 2, 512], F32, name="ot", tag="ot")
            for n, i in enumerate(i_pair):
                nc.vector.tensor_add(
                    out=ott[:, n, :], in0=pps[n], in1=x_sb[i // 2][:, i % 2, sl]
                )
            eng.dma_start(
                out=out_p[:, i_pair[0] : i_pair[0] + 2, sl], in_=ott
            )

        def emit_av_full(pr, th):
            emit_av_h(pr, th, 0, av_ps)
            emit_av_h(pr, th, 1, av_ps)
            emit_av_fin(pr, th)

        emit_av_full(0, 0)
        emit_qk_exp(3, 0)
        emit_av_full(1, 0)
        emit_qk_exp(0, 1)
        emit_av_full(2, 0)
        emit_qk_exp(1, 1)
        emit_av_full(3, 0)
        emit_qk_exp(2, 1)
        emit_av_full(0, 1)
        emit_qk_exp(3, 1)
        emit_av_full(1, 1)
        # t=0 projection: all inputs completed mid-stream; with per-half a
        # tiles there is no false WAR against the t=1 AV writes, so it can
        # fill the PE idle while av(2,1) waits on the tail exps.
        pp0a = emit_proj(0, (0, 1), [0, 1, 2, 3], True, True)
        emit_proj_out(0, (0, 1), pp0a, nc.gpsimd)
        emit_av_full(2, 1)
        pp0b = emit_proj(0, (2, 3), [0, 1, 2, 3], True, True)
        emit_proj_out(0, (2, 3), pp0b, nc.gpsimd)
        av_stack.close()
        bc_stack = ExitStack()
        av2_ps = bc_stack.enter_context(
            tc.tile_pool(name="av2_ps", bufs=1, space="PSUM")
        )
        bc_ps = bc_stack.enter_context(
            tc.tile_pool(name="bc_ps", bufs=1, space="PSUM")
        )
        # tail: last unit's AV matmuls, then t=1 partials keep the PE warm
        # under the ACT divide chain, then the j-ordered t=1 projection.
        aus31 = [emit_av_mm(3, 1, h01, av2_ps) for h01 in range(2)]
        pt_tiles.pop((3, 1))
        pp1a = emit_proj(1, (0, 1), [0, 1, 2], True, False)
        emit_av_act_div(3, 1, aus31)
        emit_proj(1, (0, 1), [3], False, True, pps=pp1a)
        emit_proj_out(1, (0, 1), pp1a, nc.scalar)
        pp1b = emit_proj(1, (2, 3), [0, 1, 2, 3], True, True)
        emit_proj_out(1, (2, 3), pp1b, nc.scalar)
        bc_stack.close()
        proj_stack.close()
        mid_stack.close()

    _split_excess_waits(nc, mybir)
    return nc


def _split_excess_waits(nc, mybir, max_waits=1):
    """This toolchain's walrus rejects engine instructions carrying more
    than one sync-wait command; hoist extras onto NoOps placed just before
    (same engine, so ordering is preserved)."""
    n_split = 0
    for fn in nc.m.functions:
        for bb in fn.blocks:
            out = []
            for inst in bb.instructions:
                si = inst.sync_info
                if si is not None and si.on_wait and len(si.on_wait) > max_waits:
                    waits = list(si.on_wait)
                    hoist, keep = waits[:-max_waits], waits[-max_waits:]
                    for k, w in enumerate(hoist):
                        nop = mybir.InstNoOp(
                            name=f"{inst.name}_hw{k}", ins=[], outs=[],
                            engine=inst.engine,
                        )
                        nop.sync_info = mybir.SyncInfo(on_wait=[w], on_update=[])
                        out.append(nop)
                    inst.sync_info = mybir.SyncInfo(
                        on_wait=keep, on_update=list(si.on_update or [])
                    )
                    n_split += 1
                out.append(inst)
            bb.instructions = out
    return n_split


def _prep_weights(gn_w, gn_b, qkv_w, qkv_b, proj_w):
    bf16 = ml_dtypes.bfloat16
    f8 = ml_dtypes.float8_e4m3
    scale = 1.0 / math.sqrt(math.sqrt(CH))
    w_eff = (qkv_w.astype(np.float64) * gn_w[None, :].astype(np.float64))
    b_eff = qkv_b.astype(np.float64) + qkv_w.astype(np.float64) @ gn_b.astype(
        np.float64
    )
    o = np.arange(3 * C)
    within = o % (3 * CH)
    rowscale = np.where(within < 2 * CH, scale, 1.0)
    w_eff = (w_eff * rowscale[:, None]).astype(np.float32)
    b_eff = (b_eff * rowscale).astype(np.float32)
    heads = np.arange(NH)[:, None] * 3 * CH
    perm_q = (heads + np.arange(CH)[None, :]).ravel()
    perm_k = (heads + CH + np.arange(CH)[None, :]).ravel()
    perm_v = (heads + 2 * CH + np.arange(CH)[None, :]).ravel()

    def dev_w8(w):  # [O, C] -> DoubleRow lhsT [128, jp, ko, O]
        # contraction channel (2*jp+ko)*128 + p
        return np.ascontiguousarray(
            w.T.reshape(2, 2, 128, w.shape[0]).transpose(2, 0, 1, 3)
        ).astype(f8)

    def dev_w(w):  # [O, C] -> bf16 lhsT chunks [128, 4, O]
        return np.ascontiguousarray(
            w.T.reshape(4, 128, w.shape[0]).transpose(1, 0, 2)
        ).astype(bf16)

    def dev_b(b):  # [512] -> [128, 4]
        return np.ascontiguousarray(b.reshape(4, 128).T).astype(np.float32)

    w8 = np.stack(
        [dev_w8(w_eff[perm_q]), dev_w8(w_eff[perm_k]), dev_w8(w_eff[perm_v])],
        axis=1,
    )
    bqk = np.concatenate([dev_b(b_eff[perm_q]), dev_b(b_eff[perm_k])], axis=1)
    g = (np.arange(128)[:, None] // 16 == np.arange(8)[None, :]).astype(np.float32)
    gg = np.zeros((128, 160), np.float32)
    gg[:, 0:8] = g / 16.0  # group-size divisor folded into the reduce matmul
    gg[:, 8:16] = bqk
    gg[0:8, 16:144] = g.T
    gg[:, 144:152] = g / (16.0 * L)  # for the ACT raw-sum stats path
    return {
        "w8": np.ascontiguousarray(w8),
        "pw": dev_w(proj_w.astype(np.float32)),
        "bv": b_eff[perm_v].reshape(1, C).astype(bf16),
        "gg": gg,
    }


def run(inputs, trace=False):
    import time

    from concourse.bass_utils import run_bass_kernel_spmd

    t0 = time.time()
    if "nc" not in _cache:
        _cache["nc"] = _build_program()
    nc = _cache["nc"]
    print(f"[kernel] program built in {time.time() - t0:.1f}s", flush=True)

    x = np.asarray(inputs["x"], dtype=np.float32)
    proj_b = np.asarray(inputs["proj_b"], dtype=np.float32)
    shared = _prep_weights(
        np.asarray(inputs["gn_w"], dtype=np.float32),
        np.asarray(inputs["gn_b"], dtype=np.float32),
        np.asarray(inputs["qkv_w"], dtype=np.float32),
        np.asarray(inputs["qkv_b"], dtype=np.float32),
        np.asarray(inputs["proj_w"], dtype=np.float32),
    )
    in_maps = []
    for b in range(NCORES):
        xb = np.ascontiguousarray(x[b].reshape(C, L))
        # residual + proj bias folded on host: out = (x + proj_b) + pw @ a
        xpb = (xb.astype(np.float64) + proj_b[:, None].astype(np.float64)).astype(
            np.float32
        )
        in_maps.append(
            {**shared, "x": xpb, "xb": xb.astype(ml_dtypes.bfloat16)}
        )
    t1 = time.time()
    res = run_bass_kernel_spmd(
        nc, in_maps, list(range(NCORES)), trace=trace,
        tmpdir=os.environ.get("BASS_KERNEL_TMPDIR"),
    )
    print(f"[kernel] executed in {time.time() - t1:.1f}s", flush=True)
    out = np.stack([res.results[b]["out"] for b in range(NCORES)])
    return out.reshape(B, C, HH, WW), res


def kernel(**inputs):
    out, _ = run(inputs)
    return out
